# revision 1
# baseline (speedup 1.0000x reference)
"""Trainium2 Bass kernel for nn_GAT_attention_multi (gnn_message_passing).

Math (per batch, N=200, D=64):
  emb = LN(embeddings); uid=emb[0], iid=emb[1], ua = uid*emb[2:]   [N,D]
  value[i,j,:] = LN(ua_i*ua_j) collapses via Gram matrices:
      mu  = (UA UA^T)/D ; var = (UA2 UA2^T)/D - mu^2 ; r = rsqrt(var+eps)
  per head: scores_ij = ua_i.vq + ua_j.vk + (cq+ck+si+ab)  (rank-2, vq=W^T aq)
      alphas = softmax_j(leaky(scores)); c = alphas*r
  out[j,d] = 0.5*sum_h [ lnw_d*(ua_jd*S1_h[j,d] - S2_h[j]) + lnb_d*S3_h[j] ]
      S1 = sum_i c_ij ua_id ; S2 = sum_i (c*mu)_ij ; S3 = sum_i alphas_ij
  row0 = uid*iid ; final leaky_relu.

Layout: feature dim d on partitions 0:64, "ones/bias" index 64. rsqrt via
exp(-0.5*ln(x)) so all ACT funcs ({Exp,Ln,Copy,Abs}) share one table.
Emission is STAGE-major across the 4 local batches so the in-order engine
queues overlap batches. Batch data-parallel: 32 batches -> 8 cores x 4.
"""

import numpy as np

import concourse.bass as bass
import concourse.tile as tile
from concourse.tile import add_dep_helper
from concourse import bacc, mybir
from concourse.masks import make_identity
from concourse.bass_utils import run_bass_kernel_spmd

AF = mybir.ActivationFunctionType
ALU = mybir.AluOpType
F32 = mybir.dt.float32

B, NODES, D = 32, 202, 64
N = NODES - 2            # 200
NCORES = 8
BL = B // NCORES         # 4 batches per core
NP = 256                 # padded N (matmul moving dim)
EPS = 1e-5
CH = [(0, 128), (128, N - 128)]  # token chunks (start, count)
SLOPE = 0.01


def build_nc():
    nc = bacc.Bacc("TRN2", target_bir_lowering=False)

    emb = nc.dram_tensor("emb", [BL, NODES, D], F32, kind="ExternalInput")
    ln_w = nc.dram_tensor("ln_w", [D], F32, kind="ExternalInput")
    ln_b = nc.dram_tensor("ln_b", [D], F32, kind="ExternalInput")
    Ws = [
        (
            nc.dram_tensor(f"W{h}_w", [D, D], F32, kind="ExternalInput"),
            nc.dram_tensor(f"W{h}_b", [D], F32, kind="ExternalInput"),
            nc.dram_tensor(f"a{h}_w", [3 * D], F32, kind="ExternalInput"),
            nc.dram_tensor(f"a{h}_b", [1], F32, kind="ExternalInput"),
        )
        for h in (1, 2)
    ]
    out = nc.dram_tensor("out", [BL, N + 1, D], F32, kind="ExternalOutput")

    with tile.TileContext(nc) as tc:
        with (
            tc.tile_pool(name="consts", bufs=1) as consts,
            tc.tile_pool(name="work", bufs=5) as work,
            tc.tile_pool(name="scr", bufs=4) as scr,
            tc.tile_pool(name="ps_tr", bufs=1, space="PSUM") as ps_tr,
            tc.tile_pool(name="ps_gh", bufs=2, space="PSUM") as ps_gh,
            tc.tile_pool(name="ps_sc", bufs=3, space="PSUM") as ps_sc,
            tc.tile_pool(name="ps_rows", bufs=2, space="PSUM") as ps_rows,
        ):
            # ---- stage A: prefetch all inputs (2 strided DMAs) ----
            eA = consts.tile([128, BL, 64], F32, tag="eA")
            nc.sync.dma_start(
                out=eA, in_=emb[:, 0:128, :].rearrange("b p d -> p b d")
            )
            eB = consts.tile([NODES - 128, BL, 64], F32, tag="eB")
            nc.sync.dma_start(
                out=eB, in_=emb[:, 128:NODES, :].rearrange("b p d -> p b d")
            )

            # ---- constants (issue spread: SP light / ACT heavy / Pool small) ----
            ident = consts.tile([128, 128], F32)
            make_identity(nc, ident)
            ones_col = consts.tile([128, 1], F32)
            nc.vector.memset(ones_col, 1.0)
            eps_t = consts.tile([128, 1], F32)
            nc.vector.memset(eps_t, EPS)
            ones2d = consts.tile([64, N], F32)
            nc.vector.memset(ones2d, 1.0)

            lnwcE = consts.tile([64, 1], F32)
            nc.sync.dma_start(out=lnwcE, in_=ln_w[:, None])
            lnbcE = consts.tile([64, 1], F32)
            nc.sync.dma_start(out=lnbcE, in_=ln_b[:, None])
            lnw_half_col = consts.tile([64, 1], F32)
            nc.scalar.mul(out=lnw_half_col, in_=lnwcE, mul=0.5)
            lnb_half_row = consts.tile([1, 65], F32)
            nc.vector.memset(lnb_half_row[0:1, 64:65], 0.0)
            nc.sync.dma_start(out=lnb_half_row[0:1, 0:64], in_=ln_b[None, :])
            lnw_row = consts.tile([1, 65], F32)
            nc.vector.memset(lnw_row[0:1, 64:65], 0.0)
            nc.sync.dma_start(out=lnw_row[0:1, 0:64], in_=ln_w[None, :])

            vqks, abts = [], []
            for h, (W_w, W_b, a_w, a_b) in enumerate(Ws):
                Wext = consts.tile([64, 65], F32, tag=f"wext{h}")
                nc.scalar.dma_start(out=Wext[:, 0:64], in_=W_w[:, :])
                nc.scalar.dma_start(out=Wext[:, 64:65], in_=W_b[:, None])
                aqk = consts.tile([64, 3], F32, tag=f"aqk{h}")
                nc.gpsimd.dma_start(
                    out=aqk, in_=a_w.rearrange("(c d) -> d c", d=64)
                )
                vqk_ps = ps_tr.tile([65, 3], F32, tag="tr")
                nc.tensor.matmul(vqk_ps, Wext, aqk, start=True, stop=True)
                vqk = consts.tile([65, 3], F32, tag=f"vqk{h}")
                nc.scalar.copy(out=vqk, in_=vqk_ps)
                vqks.append(vqk)
                abt = consts.tile([65, 1], F32, tag=f"abt{h}")
                nc.gpsimd.dma_start(out=abt[64:65, 0:1], in_=a_b[None, :])
                # fold the query-side bias cq = W_b . aq into the score const
                nc.vector.tensor_add(
                    out=abt[64:65, 0:1], in0=abt[64:65, 0:1], in1=vqk[64:65, 0:1]
                )
                abts.append(abt)

            # ---- stage B+C: layernorm, transpose, ua (per batch) ----
            elns = []
            embTs, uats, ua2ts, ua_exts = [], [], [], []
            for b in range(BL):
                eln = work.tile([128, 2, 64], F32, tag="eln")
                for c, pcnt in ((0, 128), (1, NODES - 128)):
                    src_ap = (eA if c == 0 else eB)[:pcnt, b, :]
                    st = scr.tile([128, 6], F32, tag="bnst")
                    nc.vector.bn_stats(out=st[:pcnt], in_=src_ap)
                    mv = scr.tile([128, 2], F32, tag="bnmv")
                    nc.vector.bn_aggr(out=mv[:pcnt], in_=st[:pcnt])
                    sq = scr.tile([128, 1], F32, tag="lnsq")
                    nc.scalar.activation(
                        out=sq[:pcnt], in_=mv[:pcnt, 1:2], func=AF.Sqrt,
                        bias=eps_t[:pcnt],
                    )
                    rstd = scr.tile([128, 1], F32, tag="rstd")
                    nc.vector.reciprocal_approx_fast(out=rstd[:pcnt], in_=sq[:pcnt])
                    nc.vector.tensor_scalar(
                        out=eln[:pcnt, c, :],
                        in0=src_ap,
                        scalar1=mv[:pcnt, 0:1],
                        scalar2=rstd[:pcnt],
                        op0=ALU.subtract,
                        op1=ALU.mult,
                    )
                elns.append(eln)

                # ---- stage C (merged): transpose, gamma/beta, ua, ua^2 ----
                etr = ps_tr.tile([64, NODES], F32, tag="tr")
                nc.tensor.transpose(etr[:, 0:128], elns[b][:, 0, :], ident)
                nc.tensor.transpose(
                    etr[:, 128:NODES],
                    elns[b][: NODES - 128, 1, :],
                    ident[: NODES - 128, : NODES - 128],
                )
                embT = work.tile([64, NODES], F32, tag="embT")
                nc.vector.tensor_scalar(
                    out=embT, in0=etr, scalar1=lnwcE, scalar2=lnbcE,
                    op0=ALU.mult, op1=ALU.add,
                )
                uat = work.tile([65, NP], F32, tag="uat")
                nc.gpsimd.memset(uat[0:64, N:NP], 0.0)
                nc.gpsimd.memset(uat[64:65, :], 1.0)
                nc.vector.tensor_scalar_mul(
                    out=uat[0:64, 0:N], in0=embT[:, 2:NODES], scalar1=embT[:, 0:1]
                )
                ua2t = work.tile([64, NP], F32, tag="ua2t")
                nc.gpsimd.tensor_mul(out=ua2t, in0=uat[0:64, :], in1=uat[0:64, :])
                uae = []
                for ic, (t0, cnt) in enumerate(CH):
                    utr = ps_tr.tile([128, 65], F32, tag="tr")
                    nc.tensor.transpose(
                        utr[:cnt], uat[:, t0 : t0 + cnt], ident[:65, :65]
                    )
                    u = work.tile([128, 65], F32, tag=f"uae{ic}")
                    nc.scalar.copy(out=u[:cnt], in_=utr[:cnt])
                    uae.append(u)
                embTs.append(embT)
                uats.append(uat)
                ua2ts.append(ua2t)
                ua_exts.append(uae)

            # ---- stage D+E: Gram matrices, mu/rstd maps, score rows ----
            muss, rss, Rs = [], [], []
            for b in range(BL):
                mus, rs = [], []
                for ic, (t0, cnt) in enumerate(CH):
                    gh = ps_gh.tile([128, 512], F32, tag="gh")
                    nc.tensor.matmul(
                        gh[:cnt, 0:NP],
                        uats[b][0:64, t0 : t0 + cnt],
                        uats[b][0:64, :],
                        start=True, stop=True,
                    )
                    nc.tensor.matmul(
                        gh[:cnt, NP:512],
                        ua2ts[b][:, t0 : t0 + cnt],
                        ua2ts[b],
                        start=True, stop=True,
                    )
                    mu = work.tile([128, N], F32, tag=f"mu{ic}")
                    # mu tile holds -mu (sign folded); msq=(-mu)^2 is unchanged
                    nc.scalar.mul(out=mu[:cnt], in_=gh[:cnt, 0:N], mul=-1.0 / D)
                    msq = scr.tile([128, N], F32, tag=f"msq{ic}")
                    nc.gpsimd.tensor_mul(out=msq[:cnt], in0=mu[:cnt], in1=mu[:cnt])
                    var = scr.tile([128, N], F32, tag=f"var{ic}")
                    nc.vector.scalar_tensor_tensor(
                        out=var[:cnt], in0=gh[:cnt, NP : NP + N], scalar=1.0 / D,
                        in1=msq[:cnt], op0=ALU.mult, op1=ALU.subtract,
                    )
                    sdev = scr.tile([128, N], F32, tag=f"sdev{ic}")
                    last_sqrt = nc.scalar.activation(
                        out=sdev[:cnt], in_=var[:cnt], func=AF.Sqrt,
                        bias=eps_t[:cnt],
                    )
                    r_ = work.tile([128, N], F32, tag=f"r{ic}")
                    nc.vector.reciprocal_approx_fast(out=r_[:cnt], in_=sdev[:cnt])
                    mus.append(mu)
                    rs.append(r_)
                muss.append(mus)
                rss.append(rs)

                # ---- stage E (merged): score row pieces (sk, si) and R ----
                rp = ps_rows.tile([65, 512], F32, tag="rows")
                for h in range(2):
                    nc.tensor.matmul(
                        rp[64:65, h * NP : h * NP + NP],
                        vqks[h][:, 1:2],
                        uats[b],
                        start=True, stop=True,
                    )
                iidt = scr.tile([65, 1], F32, tag="iidt")
                nc.gpsimd.memset(iidt[64:65], 1.0)
                nc.vector.tensor_copy(out=iidt[0:64], in_=embTs[b][:, 1:2])
                vi2 = scr.tile([65, 2], F32, tag="vi2")
                for h in range(2):
                    nc.vector.tensor_mul(
                        out=vi2[:, h : h + 1], in0=iidt, in1=vqks[h][:, 2:3]
                    )
                nc.tensor.matmul(
                    rp[64:65, 504:506], ones_col[0:65], vi2, start=True, stop=True
                )
                R = work.tile([65, 2 * N], F32, tag="R")
                for h in range(2):
                    nc.gpsimd.tensor_scalar_mul(
                        out=R[0:64, h * N : h * N + N],
                        in0=ones2d,
                        scalar1=vqks[h][0:64, 0:1],
                    )
                    nc.vector.tensor_scalar(
                        out=R[64:65, h * N : h * N + N],
                        in0=rp[64:65, h * NP : h * NP + N],
                        scalar1=rp[64:65, 504 + h : 505 + h],
                        scalar2=abts[h][64:65, 0:1],
                        op0=ALU.add, op1=ALU.add,
                    )
                Rs.append(R)

            # ---- stage F: scores -> leaky -> exp -> eh, c, -mu*c ----
            ehss, css, cmpss = [], [], []
            for b in range(BL):
                ehs, cs, cmps = [], [], []
                for ic, (t0, cnt) in enumerate(CH):
                    sc = ps_sc.tile([128, 2 * N], F32, tag="sc")
                    nc.tensor.matmul(
                        sc[:cnt], uats[b][:, t0 : t0 + cnt], Rs[b],
                        start=True, stop=True,
                    )
                    # leaky(x) = 0.505x + 0.495|x|  (one PSUM operand per op)
                    sabs = scr.tile([128, 2 * N], F32, tag=f"sabs{ic}")
                    nc.scalar.activation(
                        out=sabs[:cnt], in_=sc[:cnt], func=AF.Abs,
                        scale=(1.0 - SLOPE) / 2.0,
                    )
                    lr = scr.tile([128, 2 * N], F32, tag=f"lr{ic}")
                    nc.vector.scalar_tensor_tensor(
                        out=lr[:cnt], in0=sc[:cnt], scalar=(1.0 + SLOPE) / 2.0,
                        in1=sabs[:cnt], op0=ALU.mult, op1=ALU.add,
                    )
                    e = work.tile([128, 2, N], F32, tag=f"e{ic}")
                    ssum = scr.tile([128, 2], F32, tag=f"ssum{ic}")
                    for h in range(2):
                        exp_inst = nc.scalar.activation(
                            out=e[:cnt, h, :], in_=lr[:cnt, h * N : h * N + N],
                            func=AF.Exp, accum_out=ssum[:cnt, h : h + 1],
                        )
                        if b == 0 and ic == 0 and h == 0:
                            # keep all stage-D Sqrts before any Exp so the ACT
                            # func table switches exactly once
                            add_dep_helper(exp_inst.ins, last_sqrt.ins, sync=False,
                                           reason="act-table ordering")
                    rinv = scr.tile([128, 2], F32, tag=f"rinv{ic}")
                    nc.vector.reciprocal_approx_fast(out=rinv[:cnt], in_=ssum[:cnt])
                    eh = work.tile([128, 2, N], F32, tag=f"eh{ic}")
                    for h in range(2):
                        nc.scalar.mul(
                            out=eh[:cnt, h, :], in_=e[:cnt, h, :],
                            mul=rinv[:cnt, h : h + 1],
                        )
                    rrep = bass.AP(
                        tensor=rss[b][ic].tensor, offset=rss[b][ic].offset,
                        ap=[rss[b][ic].ap[0], [0, 2], [1, N]],
                    )
                    c_ = work.tile([128, 2, N], F32, tag=f"c{ic}")
                    nc.gpsimd.tensor_mul(out=c_[:cnt], in0=eh[:cnt], in1=rrep[:cnt])
                    murep = bass.AP(
                        tensor=muss[b][ic].tensor, offset=muss[b][ic].offset,
                        ap=[muss[b][ic].ap[0], [0, 2], [1, N]],
                    )
                    cmp_ = work.tile([128, 2, N], F32, tag=f"cmp{ic}")
                    nc.gpsimd.tensor_mul(
                        out=cmp_[:cnt], in0=murep[:cnt], in1=c_[:cnt]
                    )
                    ehs.append(eh)
                    cs.append(c_)
                    cmps.append(cmp_)
                ehss.append(ehs)
                css.append(cs)
                cmpss.append(cmps)

            # ---- stage G+H per batch: sums, S1^T, correction, finals ----
            osb0 = consts.tile([128, BL, 64], F32, tag="osb0")
            osb1 = consts.tile([N + 1 - 128, BL, 64], F32, tag="osb1")
            for b in range(BL):
                s3ps = ps_sc.tile([1, 2 * N], F32, tag="sc")
                for ic, (t0, cnt) in enumerate(CH):
                    nc.tensor.matmul(
                        s3ps,
                        ones_col[:cnt],
                        ehss[b][ic][:cnt].rearrange("p h j -> p (h j)"),
                        start=(ic == 0), stop=(ic == 1),
                    )
                s2ps = ps_sc.tile([1, 2 * N], F32, tag="sc")
                for ic, (t0, cnt) in enumerate(CH):
                    nc.tensor.matmul(
                        s2ps,
                        ones_col[:cnt],
                        cmpss[b][ic][:cnt].rearrange("p h j -> p (h j)"),
                        start=(ic == 0), stop=(ic == 1),
                    )
                s3_sb = scr.tile([1, 2 * N], F32, tag="s3sb")
                nc.scalar.mul(out=s3_sb, in_=s3ps, mul=0.5)
                s2_sb = scr.tile([1, 2 * N], F32, tag="s2sb")
                nc.scalar.mul(out=s2_sb, in_=s2ps, mul=0.5)

                s1t = ps_rows.tile([65, 2 * N], F32, tag="rows")
                for ic, (t0, cnt) in enumerate(CH):
                    nc.tensor.matmul(
                        s1t,
                        ua_exts[b][ic][:cnt],
                        css[b][ic][:cnt].rearrange("p h j -> p (h j)"),
                        start=(ic == 0), stop=(ic == 1),
                    )
                corr = ps_rows.tile([65, 2 * N], F32, tag="rows")
                nc.tensor.matmul(corr, lnb_half_row, s3_sb, start=True, stop=False)
                nc.tensor.matmul(corr, lnw_row, s2_sb, start=False, stop=True)

                uarep = bass.AP(
                    tensor=uats[b].tensor, offset=uats[b].offset,
                    ap=[uats[b].ap[0], [0, 2], [1, N]],
                )
                tp = scr.tile([64, 2, N], F32, tag="tp")
                nc.vector.tensor_mul(
                    out=tp, in0=s1t[0:64].rearrange("p (h j) -> p h j", h=2),
                    in1=uarep[0:64],
                )
                # 0.5*lnw applies only to the S1 term; corr already carries it
                tpc = scr.tile([64, 2, N], F32, tag="tpc")
                nc.vector.scalar_tensor_tensor(
                    out=tpc, in0=tp, scalar=lnw_half_col,
                    in1=corr[0:64].rearrange("p (h j) -> p h j", h=2),
                    op0=ALU.mult, op1=ALU.add,
                )
                outT = scr.tile([64, N + 1], F32, tag="outT")
                hs = scr.tile([64, N], F32, tag="hs")
                nc.gpsimd.tensor_add(out=hs, in0=tpc[:, 0, :], in1=tpc[:, 1, :])
                nc.vector.scalar_tensor_tensor(
                    out=outT[:, 1 : N + 1], in0=hs, scalar=SLOPE, in1=hs,
                    op0=ALU.mult, op1=ALU.max,
                )
                uii = scr.tile([64, 1], F32, tag="uii")
                nc.vector.tensor_mul(
                    out=uii, in0=embTs[b][:, 0:1], in1=embTs[b][:, 1:2]
                )
                nc.vector.scalar_tensor_tensor(
                    out=outT[:, 0:1], in0=uii, scalar=SLOPE, in1=uii,
                    op0=ALU.mult, op1=ALU.max,
                )
                for ic, (o0, ocnt) in enumerate(((0, 128), (128, N + 1 - 128))):
                    otr = ps_tr.tile([128, 64], F32, tag="tr")
                    nc.tensor.transpose(
                        otr[:ocnt], outT[:, o0 : o0 + ocnt], ident[:64, :64]
                    )
                    dst = osb0 if ic == 0 else osb1
                    nc.scalar.copy(out=dst[:ocnt, b, :], in_=otr[:ocnt])

            for b0_, b1_ in ((0, 2), (2, 4)):
                nc.sync.dma_start(
                    out=out[b0_:b1_, 0:128, :].rearrange("b p d -> p b d"),
                    in_=osb0[:, b0_:b1_, :],
                )
                nc.gpsimd.dma_start(
                    out=out[b0_:b1_, 128 : N + 1, :].rearrange("b p d -> p b d"),
                    in_=osb1[:, b0_:b1_, :],
                )

    nc.compile()
    return nc


_NC = None


def _get_nc():
    global _NC
    if _NC is None:
        _NC = build_nc()
    return _NC


def kernel(**inputs) -> np.ndarray:
    nc = _get_nc()
    emb = np.ascontiguousarray(np.asarray(inputs["embeddings"], dtype=np.float32))
    shared = {
        k: np.ascontiguousarray(np.asarray(inputs[k], np.float32))
        for k in ("ln_w", "ln_b", "W1_w", "W1_b", "a1_w", "a1_b",
                  "W2_w", "W2_b", "a2_w", "a2_b")
    }
    in_maps = [
        {"emb": np.ascontiguousarray(emb[c * BL : (c + 1) * BL]), **shared}
        for c in range(NCORES)
    ]
    res = run_bass_kernel_spmd(nc, in_maps, core_ids=list(range(NCORES)))
    return np.concatenate([res.results[c]["out"] for c in range(NCORES)], axis=0)



# revision 6
# speedup vs baseline: 1.5163x; 1.5163x over previous
"""Trainium2 Bass kernel for nn_GAT_attention_multi (gnn_message_passing).

v2 redesign vs baseline (67.6us):
  - fp32r matmuls (1 cyc/row at free>=256) and bf16 matmuls (1 cyc/row any
    size) instead of fp32 (4 cyc/row).
  - S2 folded through S1: S2_j = (1/D) sum_d ua_jd S1_dj, so the mu map, the
    c/cmp N^2 passes, and the S2/S3 row reductions all disappear. The
    lnb*S3 - lnw*S2 correction is accumulated by PE matmuls (stationary
    lnb_half const / gstat const) into the same psum bank as S1.
  - leaky_relu as one ACT Prelu op; final leaky folded into the psum->sbuf
    output copies (Prelu on ACT).
  - single ACT table switch (sqrt set -> exp set), bf16 elementwise chain
    (e, eh, ehq) on DVE at 2-4x rates.
  - host-precomputed derived constants (vq = W^T aq etc.) shipped as one
    packed params tensor: one DMA instead of ten.
  - per-batch input DMAs split across queues; 32KB contiguous loads.

Math (per batch, N=200, D=64, heads h=1,2):
  emb = LN(embeddings); ua = uid * emb[2:]           [N, D]
  G = UA UA^T, G2 = UA2 UA2^T; var = G2/D - (G/D)^2; r = rsqrt(var+eps)
  scores_ij = ua_i.vq + ua_j.vk + si + c; lr = leaky(scores)
  e = exp(lr); rinv_i = 1/sum_j e_ij; eh = e*rinv; ehq = eh*r
  S1tot[d,j] = sum_h sum_i ua_id ehq_ij            (PE, bf16)
  corr[d,j]  = sum_h lnb_half_d S3_h[j] - lnw_half_d sum_h S2_h[j]
             = sum_(h,i) lnb_half_d eh_ij  +  gstat^T tp   (PE)
      where tp = ua o S1tot, gstat[p,d] = -lnw_half_d / D
  out[j,d] = leaky( lnw_half_d * tp[d,j] + corr[d,j] ), row0 = leaky(uid*iid)
"""

import numpy as np

import concourse.bass as bass
import concourse.tile as tile
from concourse import bacc, mybir
from concourse.masks import make_identity
from concourse.bass_utils import run_bass_kernel_spmd

AF = mybir.ActivationFunctionType
ALU = mybir.AluOpType
F32 = mybir.dt.float32
F32R = mybir.dt.float32r
BF16 = mybir.dt.bfloat16

B, NODES, D = 32, 202, 64
N = NODES - 2            # 200
NCORES = 8
BL = B // NCORES         # 4 batches per core
NP = 256                 # padded N for fp32r gram moving dim
EPS = 1e-5
CH = [(0, 128), (128, N - 128)]  # i-chunks (start, count)
SLOPE = 0.01
PCOLS = 144              # packed params width


def _rep2(t, n):
    """AP view of [p, n] repeated as [p, 2, n] with stride-0 middle dim."""
    return bass.AP(tensor=t.tensor, offset=t.offset,
                   ap=[t.ap[0], [0, 2], [1, n]])


def build_nc():
    nc = bacc.Bacc("TRN2", target_bir_lowering=False)

    emb = nc.dram_tensor("emb", [BL, NODES, D], F32, kind="ExternalInput")
    par = nc.dram_tensor("par", [128, PCOLS], F32, kind="ExternalInput")
    out = nc.dram_tensor("out", [BL, N + 1, D], F32, kind="ExternalOutput")

    with tile.TileContext(nc) as tc:
        with (
            tc.tile_pool(name="consts", bufs=1) as consts,
            tc.tile_pool(name="work", bufs=3) as work,
            tc.tile_pool(name="scr", bufs=6) as scr,
            tc.tile_pool(name="ps_tr", bufs=1, space="PSUM") as ps_tr,
            tc.tile_pool(name="ps_gh", bufs=2, space="PSUM") as ps_gh,
            tc.tile_pool(name="ps_sc", bufs=2, space="PSUM") as ps_sc,
            tc.tile_pool(name="ps_s1", bufs=1, space="PSUM") as ps_s1,
        ):
            # ---------- phase 0: DMAs + consts ----------
            params = consts.tile([128, PCOLS], F32)
            nc.sync.dma_start(out=params, in_=par[:, :])

            eAs, eBs = [], []
            for b in range(BL):
                eA = consts.tile([128, D], F32, tag=f"eA{b}")
                eB = consts.tile([NODES - 128, D], F32, tag=f"eB{b}")
                eAs.append(eA)
                eBs.append(eB)
            # spread input DMAs: sync gets b0/b1, scalar b2, vector b3
            nc.sync.dma_start(out=eAs[0], in_=emb[0, 0:128, :])
            nc.sync.dma_start(out=eBs[0], in_=emb[0, 128:NODES, :])
            nc.sync.dma_start(out=eAs[1], in_=emb[1, 0:128, :])
            nc.sync.dma_start(out=eBs[1], in_=emb[1, 128:NODES, :])
            nc.scalar.dma_start(out=eAs[2], in_=emb[2, 0:128, :])
            nc.scalar.dma_start(out=eBs[2], in_=emb[2, 128:NODES, :])
            nc.scalar.dma_start(out=eAs[3], in_=emb[3, 0:128, :])
            nc.scalar.dma_start(out=eBs[3], in_=emb[3, 128:NODES, :])

            ident = consts.tile([128, 128], F32)
            make_identity(nc, ident)
            ident16 = consts.tile([128, 128], BF16)
            make_identity(nc, ident16)
            eps_t = consts.tile([128, 1], F32)
            nc.vector.memset(eps_t, EPS)
            one1 = consts.tile([1, 1], F32)
            nc.vector.memset(one1, 1.0)
            zero1 = consts.tile([1, 1], F32)
            nc.vector.memset(zero1, 0.0)
            zeroc = consts.tile([64, 1], F32)
            nc.vector.memset(zeroc, 0.0)

            lnwcE = consts.tile([64, 1], F32)
            nc.vector.tensor_copy(out=lnwcE, in_=params[0:64, 0:1])
            lnbcE = consts.tile([64, 1], F32)
            nc.vector.tensor_copy(out=lnbcE, in_=params[0:64, 1:2])

            # bf16 consts (Pool where sbuf-only)
            vksts = []
            for h in range(2):
                vkst = consts.tile([65, 128], BF16, tag=f"vkst{h}")
                nc.gpsimd.tensor_copy(
                    out=vkst, in_=params[0:65, 4 + h:5 + h].broadcast_to([65, 128]))
                vksts.append(vkst)
            ones16 = consts.tile([1, 128], BF16)
            nc.gpsimd.tensor_copy(out=ones16, in_=one1.broadcast_to([1, 128]))
            gstat16 = consts.tile([64, 64], BF16)
            nc.gpsimd.tensor_copy(out=gstat16, in_=params[0:64, 10:74])
            lnbh16 = consts.tile([128, 64], BF16)
            nc.gpsimd.tensor_copy(out=lnbh16, in_=params[:, 74:138])
            viid2 = consts.tile([64, 2], F32R)
            nc.vector.tensor_copy(out=viid2, in_=params[0:64, 6:8])

            Rvq = consts.tile([65, 400], F32R)
            for h in range(2):
                nc.gpsimd.tensor_copy(
                    out=Rvq[0:64, h * 200:(h + 1) * 200],
                    in_=params[0:64, 2 + h:3 + h].broadcast_to([64, 200]))
            nc.gpsimd.tensor_copy(out=Rvq[64:65, :],
                                  in_=zero1.broadcast_to([1, 400]))

            uats, uat16s = [], []
            for b in range(BL):
                uat = consts.tile([65, NP], F32R, tag=f"uat{b}")
                nc.gpsimd.tensor_copy(out=uat[64:65, :],
                                      in_=one1.broadcast_to([1, NP]))
                nc.gpsimd.tensor_copy(out=uat[0:64, N:NP],
                                      in_=zeroc.broadcast_to([64, NP - N]))
                uats.append(uat)
                uat16 = consts.tile([65, N], BF16, tag=f"uat16{b}")
                uat16s.append(uat16)

            osb0 = consts.tile([128, BL, 64], F32, tag="osb0")
            osb1 = consts.tile([N + 1 - 128, BL, 64], F32, tag="osb1")

            # ---------- stage BC per batch: LN, transpose, ua ----------
            embT01s, uiis, sicps = [], [], []
            for b in range(BL):
                elns = []
                for c, (src, pcnt) in enumerate(((eAs[b], 128),
                                                 (eBs[b], NODES - 128))):
                    st = scr.tile([128, 6], F32, tag="bnst")
                    nc.vector.bn_stats(out=st[:pcnt], in_=src)
                    mv = scr.tile([128, 2], F32, tag="bnmv")
                    nc.vector.bn_aggr(out=mv[:pcnt], in_=st[:pcnt])
                    sq = scr.tile([128, 1], F32, tag="lnsq")
                    nc.scalar.activation(out=sq[:pcnt], in_=mv[:pcnt, 1:2],
                                         func=AF.Sqrt, bias=eps_t[:pcnt])
                    rstd = scr.tile([128, 1], F32, tag="rstd")
                    nc.vector.reciprocal_approx_fast(out=rstd[:pcnt],
                                                     in_=sq[:pcnt])
                    eln = work.tile([128, 64], F32, tag=f"eln{c}")
                    nc.vector.tensor_scalar(
                        out=eln[:pcnt], in0=src, scalar1=mv[:pcnt, 0:1],
                        scalar2=rstd[:pcnt], op0=ALU.subtract, op1=ALU.mult)
                    elns.append(eln)

                etr = ps_tr.tile([64, NODES], F32, tag="tr")
                nc.tensor.transpose(etr[:, 0:128], elns[0], ident)
                nc.tensor.transpose(etr[:, 128:NODES],
                                    elns[1][:NODES - 128],
                                    ident[:NODES - 128, :NODES - 128])

                embT01 = consts.tile([64, 2], F32, tag=f"embT01{b}")
                nc.vector.tensor_scalar(
                    out=embT01, in0=etr[:, 0:2], scalar1=lnwcE, scalar2=lnbcE,
                    op0=ALU.mult, op1=ALU.add)
                embT01s.append(embT01)
                s1col = scr.tile([64, 1], F32, tag="s1col")
                nc.vector.tensor_mul(out=s1col, in0=embT01[:, 0:1], in1=lnwcE)
                s2col = scr.tile([64, 1], F32, tag="s2col")
                nc.vector.tensor_mul(out=s2col, in0=embT01[:, 0:1], in1=lnbcE)
                # ua_dj = s1_d * eln_items + s2_d  (gamma/beta + uid folded)
                nc.vector.tensor_scalar(
                    out=uats[b][0:64, 0:N], in0=etr[:, 2:NODES],
                    scalar1=s1col, scalar2=s2col, op0=ALU.mult, op1=ALU.add)
                nc.gpsimd.tensor_copy(out=uat16s[b], in_=uats[b][:, 0:N])

                uii = scr.tile([64, 1], F32, tag="uii")
                nc.vector.tensor_mul(out=uii, in0=embT01[:, 0:1],
                                     in1=embT01[:, 1:2])
                uiis.append(uii)

            # ---------- stage D per batch: gram, var, r ----------
            ua2ts, r16s = [], []
            for b in range(BL):
                ua2t = work.tile([64, NP], F32R, tag="ua2t")
                nc.gpsimd.tensor_mul(out=ua2t, in0=uats[b][0:64, :],
                                     in1=uats[b][0:64, :])
                ua2ts.append(ua2t)
                r16b = []
                for c, (t0, cnt) in enumerate(CH):
                    gh = ps_gh.tile([128, 512], F32, tag="gh")
                    nc.tensor.matmul(gh[:cnt, 0:NP],
                                     uats[b][0:64, t0:t0 + cnt],
                                     uats[b][0:64, :], start=True, stop=True)
                    nc.tensor.matmul(gh[:cnt, NP:512],
                                     ua2t[:, t0:t0 + cnt],
                                     ua2t, start=True, stop=True)
                    msq = scr.tile([128, N], F32, tag=f"msq{c}")
                    nc.scalar.activation(out=msq[:cnt], in_=gh[:cnt, 0:N],
                                         func=AF.Square, scale=1.0 / D)
                    var = scr.tile([128, N], F32, tag=f"var{c}")
                    nc.vector.scalar_tensor_tensor(
                        out=var[:cnt], in0=gh[:cnt, NP:NP + N],
                        scalar=1.0 / D, in1=msq[:cnt],
                        op0=ALU.mult, op1=ALU.subtract)
                    sdev = scr.tile([128, N], F32, tag=f"sdev{c}")
                    nc.scalar.activation(out=sdev[:cnt], in_=var[:cnt],
                                         func=AF.Sqrt, bias=eps_t[:cnt])
                    r32 = scr.tile([128, N], F32, tag=f"r32{c}")
                    nc.vector.reciprocal_approx_fast(out=r32[:cnt],
                                                     in_=sdev[:cnt])
                    r16 = work.tile([128, N], BF16, tag=f"r16{c}")
                    nc.gpsimd.tensor_copy(out=r16[:cnt], in_=r32[:cnt])
                    r16b.append(r16)
                r16s.append(r16b)

            # ---------- stage E per batch: si const + sib row ----------
            for b in range(BL):
                iidr = scr.tile([64, 1], F32R, tag="iidr")
                nc.vector.tensor_copy(out=iidr, in_=embT01s[b][:, 1:2])
                si_ps = ps_tr.tile([1, 2], F32, tag="tr")
                nc.tensor.matmul(si_ps, iidr, viid2, start=True, stop=True)
                sicp = scr.tile([1, 2], F32, tag="sicp")
                nc.vector.tensor_add(out=sicp, in0=si_ps,
                                     in1=params[0:1, 138:140])
                sib16 = work.tile([1, 400], BF16, tag="sib16")
                for h in range(2):
                    nc.vector.tensor_copy(
                        out=sib16[0:1, h * 200:(h + 1) * 200],
                        in_=sicp[0:1, h:h + 1].broadcast_to([1, 200]))
                sicps.append(sib16)

            # ---------- stage F + G per batch ----------
            for b in range(BL):
                ehs, ehqs = [], []
                s1c = ps_s1.tile([128, N], F32, tag="s1c")
                for c, (t0, cnt) in enumerate(CH):
                    sc = ps_sc.tile([128, 400], F32, tag="sc")
                    nc.tensor.matmul(sc[:cnt], uats[b][:, t0:t0 + cnt],
                                     Rvq, start=True, stop=False)
                    for h in range(2):
                        nc.tensor.matmul(
                            sc[:cnt, h * 200:(h + 1) * 200],
                            vksts[h][:, 0:cnt], uat16s[b],
                            start=False, stop=False)
                    nc.tensor.matmul(sc[:cnt], ones16[0:1, 0:cnt],
                                     sicps[b], start=False, stop=True)

                    lr = work.tile([128, 400], F32, tag=f"lr{c}")
                    nc.scalar.activation(out=lr[:cnt], in_=sc[:cnt],
                                         func=AF.Prelu, alpha=SLOPE)
                    e = work.tile([128, 2, N], BF16, tag=f"e{c}")
                    ssum = scr.tile([128, 2], F32, tag=f"ssum{c}")
                    for h in range(2):
                        nc.scalar.activation(
                            out=e[:cnt, h, :],
                            in_=lr[:cnt, h * 200:(h + 1) * 200],
                            func=AF.Exp, accum_out=ssum[:cnt, h:h + 1])
                    rinv = scr.tile([128, 2], F32, tag=f"rinv{c}")
                    nc.vector.reciprocal_approx_fast(out=rinv[:cnt],
                                                     in_=ssum[:cnt])
                    eh = work.tile([128, 2, N], BF16, tag=f"eh{c}")
                    for h in range(2):
                        nc.vector.tensor_scalar_mul(
                            out=eh[:cnt, h, :], in0=e[:cnt, h, :],
                            scalar1=rinv[:cnt, h:h + 1])
                    ehq = work.tile([128, 2, N], BF16, tag=f"ehq{c}")
                    nc.vector.tensor_mul(out=ehq[:cnt], in0=eh[:cnt],
                                         in1=_rep2(r16s[b][c], N)[:cnt])
                    ehs.append(eh)
                    ehqs.append(ehq)

                    # ua_ext (bf16) for this chunk via bf16 transpose
                    uaet = ps_tr.tile([128, 64], BF16, tag="tr16")
                    nc.tensor.transpose(uaet[:cnt],
                                        uat16s[b][0:64, t0:t0 + cnt],
                                        ident16[0:64, 0:64])
                    uaexb = work.tile([128, 64], BF16, tag=f"uaexb{c}")
                    nc.vector.tensor_copy(out=uaexb[:cnt], in_=uaet[:cnt])

                    first = c == 0
                    for h in range(2):
                        nc.tensor.matmul(
                            s1c[0:64, :], uaexb[:cnt], ehq[:cnt, h, :],
                            start=(first and h == 0),
                            stop=(c == 1 and h == 1))
                    for h in range(2):
                        nc.tensor.matmul(
                            s1c[64:128, :], lnbh16[0:cnt, :], eh[:cnt, h, :],
                            start=(first and h == 0), stop=False)

                # ---- stage G ----
                tp32 = work.tile([64, N], F32, tag="tp32")
                nc.vector.tensor_mul(out=tp32, in0=s1c[0:64, :],
                                     in1=uats[b][0:64, 0:N])
                tp16 = work.tile([64, N], BF16, tag="tp16")
                nc.gpsimd.tensor_copy(out=tp16, in_=tp32)
                nc.tensor.matmul(s1c[64:128, :], gstat16, tp16,
                                 start=False, stop=True)

                outT = work.tile([64, N + 1], F32, tag="outT")
                nc.vector.tensor_copy(out=outT[:, 0:1], in_=uiis[b])
                nc.vector.scalar_tensor_tensor(
                    out=outT[:, 1:N + 1], in0=tp32,
                    scalar=params[0:64, 9:10], in1=s1c[64:128, :],
                    op0=ALU.mult, op1=ALU.add)

                for c, (o0, ocnt) in enumerate(((0, 128), (128, N + 1 - 128))):
                    otr = ps_tr.tile([128, 64], F32, tag="otr")
                    nc.tensor.transpose(otr[:ocnt], outT[:, o0:o0 + ocnt],
                                        ident[0:64, 0:64])
                    dst = osb0 if c == 0 else osb1
                    nc.scalar.activation(out=dst[:ocnt, b, :], in_=otr[:ocnt],
                                         func=AF.Prelu, alpha=SLOPE)
                qs = [nc.sync, nc.scalar, nc.sync, nc.scalar]
                qs[b % 2].dma_start(out=out[b, 0:128, :], in_=osb0[:, b, :])
                qs[(b + 1) % 2].dma_start(out=out[b, 128:N + 1, :],
                                          in_=osb1[:, b, :])

    nc.compile()
    return nc


_NC = None


def _get_nc():
    global _NC
    if _NC is None:
        _NC = build_nc()
    return _NC


def _pack_params(inputs):
    f = lambda k: np.asarray(inputs[k], np.float32)
    ln_w, ln_b = f("ln_w"), f("ln_b")
    p = np.zeros((128, PCOLS), np.float32)
    p[0:64, 0] = ln_w
    p[0:64, 1] = ln_b
    for h, (W, Wb, aw, ab) in enumerate(
            ((f("W1_w"), f("W1_b"), f("a1_w"), f("a1_b")),
             (f("W2_w"), f("W2_b"), f("a2_w"), f("a2_b")))):
        aq, ak, ai = aw[0:64], aw[64:128], aw[128:192]
        p[0:64, 2 + h] = W.T @ aq           # vq
        p[0:64, 4 + h] = W.T @ ak           # vk
        p[64, 4 + h] = Wb @ ak              # key-side bias
        p[0:64, 6 + h] = W.T @ ai           # viid
        p[0, 138 + h] = Wb @ aq + Wb @ ai + ab[0]   # cq + iid bias + ab
    lnw_half = 0.5 * ln_w
    lnb_half = 0.5 * ln_b
    p[0:64, 9] = lnw_half
    p[0:64, 10:74] = np.tile((-lnw_half / D)[None, :], (64, 1))
    p[:, 74:138] = np.tile(lnb_half[None, :], (128, 1))
    p[0:64, 8] = lnb_half
    return np.ascontiguousarray(p)


def make_in_maps(inputs):
    emb = np.ascontiguousarray(np.asarray(inputs["embeddings"], np.float32))
    p = _pack_params(inputs)
    return [
        {"emb": np.ascontiguousarray(emb[c * BL:(c + 1) * BL]), "par": p}
        for c in range(NCORES)
    ]


def kernel(**inputs) -> np.ndarray:
    nc = _get_nc()
    in_maps = make_in_maps(inputs)
    res = run_bass_kernel_spmd(nc, in_maps, core_ids=list(range(NCORES)))
    return np.concatenate([res.results[c]["out"] for c in range(NCORES)],
                          axis=0)


# revision 7
# speedup vs baseline: 1.6358x; 1.0788x over previous
"""Trainium2 Bass kernel for nn_GAT_attention_multi (gnn_message_passing).

v2 redesign vs baseline (67.6us):
  - fp32r matmuls (1 cyc/row at free>=256) and bf16 matmuls (1 cyc/row any
    size) instead of fp32 (4 cyc/row).
  - S2 folded through S1: S2_j = (1/D) sum_d ua_jd S1_dj, so the mu map, the
    c/cmp N^2 passes, and the S2/S3 row reductions all disappear. The
    lnb*S3 - lnw*S2 correction is accumulated by PE matmuls (stationary
    lnb_half const / gstat const) into the same psum bank as S1.
  - leaky_relu as one ACT Prelu op; final leaky folded into the psum->sbuf
    output copies (Prelu on ACT).
  - single ACT table switch (sqrt set -> exp set), bf16 elementwise chain
    (e, eh, ehq) on DVE at 2-4x rates.
  - host-precomputed derived constants (vq = W^T aq etc.) shipped as one
    packed params tensor: one DMA instead of ten.
  - per-batch input DMAs split across queues; 32KB contiguous loads.

Math (per batch, N=200, D=64, heads h=1,2):
  emb = LN(embeddings); ua = uid * emb[2:]           [N, D]
  G = UA UA^T, G2 = UA2 UA2^T; var = G2/D - (G/D)^2; r = rsqrt(var+eps)
  scores_ij = ua_i.vq + ua_j.vk + si + c; lr = leaky(scores)
  e = exp(lr); rinv_i = 1/sum_j e_ij; eh = e*rinv; ehq = eh*r
  S1tot[d,j] = sum_h sum_i ua_id ehq_ij            (PE, bf16)
  corr[d,j]  = sum_h lnb_half_d S3_h[j] - lnw_half_d sum_h S2_h[j]
             = sum_(h,i) lnb_half_d eh_ij  +  gstat^T tp   (PE)
      where tp = ua o S1tot, gstat[p,d] = -lnw_half_d / D
  out[j,d] = leaky( lnw_half_d * tp[d,j] + corr[d,j] ), row0 = leaky(uid*iid)
"""

import numpy as np

import concourse.bass as bass
import concourse.tile as tile
from concourse import bacc, mybir
from concourse.masks import make_identity
from concourse.bass_utils import run_bass_kernel_spmd

AF = mybir.ActivationFunctionType
ALU = mybir.AluOpType
F32 = mybir.dt.float32
F32R = mybir.dt.float32r
BF16 = mybir.dt.bfloat16

B, NODES, D = 32, 202, 64
N = NODES - 2            # 200
NCORES = 8
BL = B // NCORES         # 4 batches per core
NP = 256                 # padded N for fp32r gram moving dim
EPS = 1e-5
CH = [(0, 128), (128, N - 128)]  # i-chunks (start, count)
SLOPE = 0.01
PCOLS = 144              # packed params width


def _rep2(t, n):
    """AP view of [p, n] repeated as [p, 2, n] with stride-0 middle dim."""
    return bass.AP(tensor=t.tensor, offset=t.offset,
                   ap=[t.ap[0], [0, 2], [1, n]])


def build_nc():
    nc = bacc.Bacc("TRN2", target_bir_lowering=False)

    emb = nc.dram_tensor("emb", [BL, NODES, D], F32, kind="ExternalInput")
    par = nc.dram_tensor("par", [128, PCOLS], F32, kind="ExternalInput")
    out = nc.dram_tensor("out", [BL, N + 1, D], F32, kind="ExternalOutput")

    with tile.TileContext(nc) as tc:
        with (
            tc.tile_pool(name="consts", bufs=1) as consts,
            tc.tile_pool(name="work", bufs=3) as work,
            tc.tile_pool(name="scr", bufs=6) as scr,
            tc.tile_pool(name="ps_tr", bufs=1, space="PSUM") as ps_tr,
            tc.tile_pool(name="ps_gh", bufs=2, space="PSUM") as ps_gh,
            tc.tile_pool(name="ps_sc", bufs=2, space="PSUM") as ps_sc,
            tc.tile_pool(name="ps_s1", bufs=1, space="PSUM") as ps_s1,
        ):
            # ---------- phase 0: DMAs + consts ----------
            params = consts.tile([128, PCOLS], F32)
            nc.sync.dma_start(out=params, in_=par[:, :])

            eAt = consts.tile([128, BL, D], F32, tag="eAt")
            nc.scalar.dma_start(
                out=eAt, in_=emb[:, 0:128, :].rearrange("b p d -> p b d"))
            eBt = consts.tile([NODES - 128, BL, D], F32, tag="eBt")
            nc.sync.dma_start(
                out=eBt, in_=emb[:, 128:NODES, :].rearrange("b p d -> p b d"))
            eAs = [eAt[:, b, :] for b in range(BL)]
            eBs = [eBt[:, b, :] for b in range(BL)]

            ident = consts.tile([128, 128], F32)
            make_identity(nc, ident)
            ident16 = consts.tile([128, 128], BF16)
            make_identity(nc, ident16)
            eps_t = consts.tile([128, 1], F32)
            nc.vector.memset(eps_t, EPS)
            warm = consts.tile([1, 1], F32)
            nc.scalar.activation(out=warm, in_=eps_t[0:1], func=AF.Sqrt)
            one1 = consts.tile([1, 1], F32)
            nc.vector.memset(one1, 1.0)
            zero1 = consts.tile([1, 1], F32)
            nc.vector.memset(zero1, 0.0)
            zeroc = consts.tile([64, 1], F32)
            nc.vector.memset(zeroc, 0.0)

            lnwcE = consts.tile([64, 1], F32)
            nc.vector.tensor_copy(out=lnwcE, in_=params[0:64, 0:1])
            lnbcE = consts.tile([64, 1], F32)
            nc.vector.tensor_copy(out=lnbcE, in_=params[0:64, 1:2])

            # bf16 consts (Pool where sbuf-only)
            vksts = []
            for h in range(2):
                vkst = consts.tile([65, 128], BF16, tag=f"vkst{h}")
                nc.gpsimd.tensor_copy(
                    out=vkst, in_=params[0:65, 4 + h:5 + h].broadcast_to([65, 128]))
                vksts.append(vkst)
            ones16 = consts.tile([1, 128], BF16)
            nc.gpsimd.tensor_copy(out=ones16, in_=one1.broadcast_to([1, 128]))
            gstat16 = consts.tile([64, 64], BF16)
            nc.gpsimd.tensor_copy(out=gstat16, in_=params[0:64, 10:74])
            lnbh16 = consts.tile([128, 64], BF16)
            nc.gpsimd.tensor_copy(out=lnbh16, in_=params[:, 74:138])
            viid2 = consts.tile([64, 2], F32R)
            nc.vector.tensor_copy(out=viid2, in_=params[0:64, 6:8])

            Rvq = consts.tile([65, 400], F32R)
            for h in range(2):
                nc.gpsimd.tensor_copy(
                    out=Rvq[0:64, h * 200:(h + 1) * 200],
                    in_=params[0:64, 2 + h:3 + h].broadcast_to([64, 200]))
            nc.gpsimd.tensor_copy(out=Rvq[64:65, :],
                                  in_=zero1.broadcast_to([1, 400]))

            uats, uat16s = [], []
            for b in range(BL):
                uat = consts.tile([65, NP], F32R, tag=f"uat{b}")
                nc.gpsimd.tensor_copy(out=uat[64:65, :],
                                      in_=one1.broadcast_to([1, NP]))
                nc.gpsimd.tensor_copy(out=uat[0:64, N:NP],
                                      in_=zeroc.broadcast_to([64, NP - N]))
                uats.append(uat)
                uat16 = consts.tile([65, N], BF16, tag=f"uat16{b}")
                uat16s.append(uat16)

            osb0 = consts.tile([128, BL, 64], F32, tag="osb0")
            osb1 = consts.tile([N + 1 - 128, BL, 64], F32, tag="osb1")

            # ---------- stage BC per batch: LN, transpose, ua ----------
            embT01s, uiis, sicps = [], [], []
            for b in range(BL):
                elns = []
                for c, (src, pcnt) in enumerate(((eAs[b], 128),
                                                 (eBs[b], NODES - 128))):
                    st = scr.tile([128, 6], F32, tag="bnst")
                    nc.vector.bn_stats(out=st[:pcnt], in_=src)
                    mv = scr.tile([128, 2], F32, tag="bnmv")
                    nc.vector.bn_aggr(out=mv[:pcnt], in_=st[:pcnt])
                    sq = scr.tile([128, 1], F32, tag="lnsq")
                    nc.scalar.activation(out=sq[:pcnt], in_=mv[:pcnt, 1:2],
                                         func=AF.Sqrt, bias=eps_t[:pcnt])
                    rstd = scr.tile([128, 1], F32, tag="rstd")
                    nc.vector.reciprocal_approx_fast(out=rstd[:pcnt],
                                                     in_=sq[:pcnt])
                    eln = work.tile([128, 64], F32, tag=f"eln{c}")
                    nc.vector.tensor_scalar(
                        out=eln[:pcnt], in0=src, scalar1=mv[:pcnt, 0:1],
                        scalar2=rstd[:pcnt], op0=ALU.subtract, op1=ALU.mult)
                    elns.append(eln)

                etr = ps_tr.tile([64, NODES], F32, tag="tr")
                nc.tensor.transpose(etr[:, 0:128], elns[0], ident)
                nc.tensor.transpose(etr[:, 128:NODES],
                                    elns[1][:NODES - 128],
                                    ident[:NODES - 128, :NODES - 128])

                embT01 = consts.tile([64, 2], F32, tag=f"embT01{b}")
                nc.vector.tensor_scalar(
                    out=embT01, in0=etr[:, 0:2], scalar1=lnwcE, scalar2=lnbcE,
                    op0=ALU.mult, op1=ALU.add)
                embT01s.append(embT01)
                s1col = scr.tile([64, 1], F32, tag="s1col")
                nc.vector.tensor_mul(out=s1col, in0=embT01[:, 0:1], in1=lnwcE)
                s2col = scr.tile([64, 1], F32, tag="s2col")
                nc.vector.tensor_mul(out=s2col, in0=embT01[:, 0:1], in1=lnbcE)
                # ua_dj = s1_d * eln_items + s2_d  (gamma/beta + uid folded)
                nc.vector.tensor_scalar(
                    out=uats[b][0:64, 0:N], in0=etr[:, 2:NODES],
                    scalar1=s1col, scalar2=s2col, op0=ALU.mult, op1=ALU.add)
                nc.gpsimd.tensor_copy(out=uat16s[b], in_=uats[b][:, 0:N])

                uii = scr.tile([64, 1], F32, tag="uii")
                nc.vector.tensor_mul(out=uii, in0=embT01[:, 0:1],
                                     in1=embT01[:, 1:2])
                uiis.append(uii)

            # ---------- stage D per batch: gram, var, r ----------
            ua2ts, r16s = [], []
            for b in range(BL):
                ua2t = work.tile([64, NP], F32R, tag="ua2t")
                nc.gpsimd.tensor_mul(out=ua2t, in0=uats[b][0:64, :],
                                     in1=uats[b][0:64, :])
                ua2ts.append(ua2t)
                r16b = []
                for c, (t0, cnt) in enumerate(CH):
                    gh = ps_gh.tile([128, 512], F32, tag="gh")
                    nc.tensor.matmul(gh[:cnt, 0:NP],
                                     uats[b][0:64, t0:t0 + cnt],
                                     uats[b][0:64, :], start=True, stop=True)
                    nc.tensor.matmul(gh[:cnt, NP:512],
                                     ua2t[:, t0:t0 + cnt],
                                     ua2t, start=True, stop=True)
                    msq = scr.tile([128, N], F32, tag=f"msq{c}")
                    nc.scalar.activation(out=msq[:cnt], in_=gh[:cnt, 0:N],
                                         func=AF.Square, scale=1.0 / D)
                    var = scr.tile([128, N], F32, tag=f"var{c}")
                    nc.vector.scalar_tensor_tensor(
                        out=var[:cnt], in0=gh[:cnt, NP:NP + N],
                        scalar=1.0 / D, in1=msq[:cnt],
                        op0=ALU.mult, op1=ALU.subtract)
                    sdev = scr.tile([128, N], F32, tag=f"sdev{c}")
                    nc.scalar.activation(out=sdev[:cnt], in_=var[:cnt],
                                         func=AF.Sqrt, bias=eps_t[:cnt])
                    r32 = scr.tile([128, N], F32, tag=f"r32{c}")
                    nc.vector.reciprocal_approx_fast(out=r32[:cnt],
                                                     in_=sdev[:cnt])
                    r16 = work.tile([128, N], BF16, tag=f"r16{c}")
                    nc.gpsimd.tensor_copy(out=r16[:cnt], in_=r32[:cnt])
                    r16b.append(r16)
                r16s.append(r16b)

            # ---------- stage E per batch: si const + sib row ----------
            for b in range(BL):
                iidr = scr.tile([64, 1], F32R, tag="iidr")
                nc.vector.tensor_copy(out=iidr, in_=embT01s[b][:, 1:2])
                si_ps = ps_tr.tile([1, 2], F32, tag="tr")
                nc.tensor.matmul(si_ps, iidr, viid2, start=True, stop=True)
                sicp = scr.tile([1, 2], F32, tag="sicp")
                nc.vector.tensor_add(out=sicp, in0=si_ps,
                                     in1=params[0:1, 138:140])
                sib16 = work.tile([1, 400], BF16, tag="sib16")
                for h in range(2):
                    nc.vector.tensor_copy(
                        out=sib16[0:1, h * 200:(h + 1) * 200],
                        in_=sicp[0:1, h:h + 1].broadcast_to([1, 200]))
                sicps.append(sib16)

            # ---------- stage F + G per batch ----------
            for b in range(BL):
                ehs, ehqs = [], []
                s1c = ps_s1.tile([128, N], F32, tag="s1c")
                for c, (t0, cnt) in enumerate(CH):
                    sc = ps_sc.tile([128, 400], F32, tag="sc")
                    nc.tensor.matmul(sc[:cnt], uats[b][:, t0:t0 + cnt],
                                     Rvq, start=True, stop=False)
                    for h in range(2):
                        nc.tensor.matmul(
                            sc[:cnt, h * 200:(h + 1) * 200],
                            vksts[h][:, 0:cnt], uat16s[b],
                            start=False, stop=False)
                    nc.tensor.matmul(sc[:cnt], ones16[0:1, 0:cnt],
                                     sicps[b], start=False, stop=True)

                    lr = work.tile([128, 400], F32, tag=f"lr{c}")
                    nc.scalar.activation(out=lr[:cnt], in_=sc[:cnt],
                                         func=AF.Prelu, alpha=SLOPE)
                    e = work.tile([128, 2, N], BF16, tag=f"e{c}")
                    ssum = scr.tile([128, 2], F32, tag=f"ssum{c}")
                    for h in range(2):
                        nc.scalar.activation(
                            out=e[:cnt, h, :],
                            in_=lr[:cnt, h * 200:(h + 1) * 200],
                            func=AF.Exp, accum_out=ssum[:cnt, h:h + 1])
                    rinv = scr.tile([128, 2], F32, tag=f"rinv{c}")
                    nc.vector.reciprocal_approx_fast(out=rinv[:cnt],
                                                     in_=ssum[:cnt])
                    eh = work.tile([128, 2, N], BF16, tag=f"eh{c}")
                    for h in range(2):
                        nc.vector.tensor_scalar_mul(
                            out=eh[:cnt, h, :], in0=e[:cnt, h, :],
                            scalar1=rinv[:cnt, h:h + 1])
                    ehq = work.tile([128, 2, N], BF16, tag=f"ehq{c}")
                    nc.vector.tensor_mul(out=ehq[:cnt], in0=eh[:cnt],
                                         in1=_rep2(r16s[b][c], N)[:cnt])
                    ehs.append(eh)
                    ehqs.append(ehq)

                    # ua_ext (bf16) for this chunk via bf16 transpose
                    uaet = ps_tr.tile([128, 64], BF16, tag="tr16")
                    nc.tensor.transpose(uaet[:cnt],
                                        uat16s[b][0:64, t0:t0 + cnt],
                                        ident16[0:64, 0:64])
                    uaexb = work.tile([128, 64], BF16, tag=f"uaexb{c}")
                    nc.vector.tensor_copy(out=uaexb[:cnt], in_=uaet[:cnt])

                    first = c == 0
                    for h in range(2):
                        nc.tensor.matmul(
                            s1c[0:64, :], uaexb[:cnt], ehq[:cnt, h, :],
                            start=(first and h == 0),
                            stop=(c == 1 and h == 1))
                    for h in range(2):
                        nc.tensor.matmul(
                            s1c[64:128, :], lnbh16[0:cnt, :], eh[:cnt, h, :],
                            start=(first and h == 0), stop=False)

                # ---- stage G ----
                tp32 = work.tile([64, N], F32, tag="tp32")
                nc.vector.tensor_mul(out=tp32, in0=s1c[0:64, :],
                                     in1=uats[b][0:64, 0:N])
                tp16 = work.tile([64, N], BF16, tag="tp16")
                nc.gpsimd.tensor_copy(out=tp16, in_=tp32)
                nc.tensor.matmul(s1c[64:128, :], gstat16, tp16,
                                 start=False, stop=True)

                outT = work.tile([64, N + 1], F32, tag="outT")
                nc.vector.tensor_copy(out=outT[:, 0:1], in_=uiis[b])
                nc.vector.scalar_tensor_tensor(
                    out=outT[:, 1:N + 1], in0=tp32,
                    scalar=params[0:64, 9:10], in1=s1c[64:128, :],
                    op0=ALU.mult, op1=ALU.add)

                for c, (o0, ocnt) in enumerate(((0, 128), (128, N + 1 - 128))):
                    otr = ps_tr.tile([128, 64], F32, tag="otr")
                    nc.tensor.transpose(otr[:ocnt], outT[:, o0:o0 + ocnt],
                                        ident[0:64, 0:64])
                    dst = osb0 if c == 0 else osb1
                    nc.scalar.activation(out=dst[:ocnt, b, :], in_=otr[:ocnt],
                                         func=AF.Prelu, alpha=SLOPE)
                qs = [nc.sync, nc.scalar, nc.sync, nc.scalar]
                qs[b % 2].dma_start(out=out[b, 0:128, :], in_=osb0[:, b, :])
                qs[(b + 1) % 2].dma_start(out=out[b, 128:N + 1, :],
                                          in_=osb1[:, b, :])

    nc.compile()
    return nc


_NC = None


def _get_nc():
    global _NC
    if _NC is None:
        _NC = build_nc()
    return _NC


def _pack_params(inputs):
    f = lambda k: np.asarray(inputs[k], np.float32)
    ln_w, ln_b = f("ln_w"), f("ln_b")
    p = np.zeros((128, PCOLS), np.float32)
    p[0:64, 0] = ln_w
    p[0:64, 1] = ln_b
    for h, (W, Wb, aw, ab) in enumerate(
            ((f("W1_w"), f("W1_b"), f("a1_w"), f("a1_b")),
             (f("W2_w"), f("W2_b"), f("a2_w"), f("a2_b")))):
        aq, ak, ai = aw[0:64], aw[64:128], aw[128:192]
        p[0:64, 2 + h] = W.T @ aq           # vq
        p[0:64, 4 + h] = W.T @ ak           # vk
        p[64, 4 + h] = Wb @ ak              # key-side bias
        p[0:64, 6 + h] = W.T @ ai           # viid
        p[0, 138 + h] = Wb @ aq + Wb @ ai + ab[0]   # cq + iid bias + ab
    lnw_half = 0.5 * ln_w
    lnb_half = 0.5 * ln_b
    p[0:64, 9] = lnw_half
    p[0:64, 10:74] = np.tile((-lnw_half / D)[None, :], (64, 1))
    p[:, 74:138] = np.tile(lnb_half[None, :], (128, 1))
    p[0:64, 8] = lnb_half
    return np.ascontiguousarray(p)


def make_in_maps(inputs):
    emb = np.ascontiguousarray(np.asarray(inputs["embeddings"], np.float32))
    p = _pack_params(inputs)
    return [
        {"emb": np.ascontiguousarray(emb[c * BL:(c + 1) * BL]), "par": p}
        for c in range(NCORES)
    ]


def kernel(**inputs) -> np.ndarray:
    nc = _get_nc()
    in_maps = make_in_maps(inputs)
    res = run_bass_kernel_spmd(nc, in_maps, core_ids=list(range(NCORES)))
    return np.concatenate([res.results[c]["out"] for c in range(NCORES)],
                          axis=0)
